# revision 1
# baseline (speedup 1.0000x reference)
"""Trainium2 Bass kernel v2.1 for nn_LossFunction_49615462203607.

Fused single-P design (see kernel_v2): each core computes
P[i=tgt-half(2048), j=src(4096)] for its batch once; rowmin -> loss_2
partial, colacc+transpose+ReduceScatter(min) -> loss_1 shard.

v2.1 engine split per group [128 i, 2048 j] (one (half, tile) step):
  PE:  4x matmul [27,128]x[27,512] -> PSUM f32 (bf16 split-K augment)
  Sc:  ACTIVATE copy PSUM f32 -> SBUF fp16 (the only PSUM extraction)
  DVE: fold TT 2048->1024->512 (fp16 2x) for rowmin, one CACHE_REDUCE-512
       per step, one TT-2048 colacc min (first step per range = copy)
All big constants DMA'd from host (no large DVE memsets). Tail per j-half:
PE transposes colacc into a bitcast-fp16 PSUM slot, DVE reduce -> [128,16]
partial colmins, pair ReduceScatter(min) -> each core hubers its [64,16]
shard (no double-count). PSUM: 2x4-bank group buffers, tails reuse slots.
"""

import os
from contextlib import ExitStack

import numpy as np

import concourse.bacc as bacc
import concourse.bass as bass
import concourse.mybir as mybir
import concourse.tile as tile
from concourse import bass_utils


F32 = mybir.dt.float32
F16 = mybir.dt.float16
BF16 = mybir.dt.bfloat16
ALU = mybir.AluOpType

B = 4
N = 4096
KP = 512
MARGIN = 0.01
NCORES = 8
HALF = N // 2
IT = 128
N_ITILES = HALF // IT        # 16
GW = 2048                    # j columns per group (4 psum banks)
KP_H = KP // 2
K_ROWS = 27

_BUILD_CACHE: dict = {}


def _stage_x(nc, pools, x_dram, xa_c):
    """xa row blocks (9 rows each): [x^2_h,1,x_h | x^2_l,0,x_l | x^2_h,1,x_h].
    Const rows come straight from DRAM; hi data rows write blocks {0,2} in
    one partition-strided DMA each."""
    aug, s96 = pools["aug"], pools["stage"]
    xa = aug.tile([K_ROWS, HALF], BF16, name="xa")
    xw = HALF // 4
    xq = s96.tile([12, xw], F32, tag="xq", name="xq")
    nc.sync.dma_start(out=xq, in_=x_dram.rearrange("d (c f) -> d c f", c=4))
    nc.gpsimd.dma_start(xa[3:6, :], xa_c[0:3, :])
    nc.gpsimd.dma_start(xa[12:15, :], xa_c[3:6, :])
    nc.gpsimd.dma_start(xa[21:24, :], xa_c[6:9, :])
    xsq = s96.tile([12, xw], F32, tag="xsq", name="xsq")
    nc.scalar.square(xsq, xq)
    sq_h = s96.tile([12, xw], BF16, tag="xsqh", name="xsqh")
    nc.scalar.copy(sq_h, xsq)
    sq_l = s96.tile([12, xw], BF16, tag="xsql", name="xsql")
    nc.vector.tensor_sub(sq_l, xsq, sq_h)
    raw_h = s96.tile([12, xw], BF16, tag="xrawh", name="xrawh")
    nc.scalar.copy(raw_h, xq)
    raw_l = s96.tile([12, xw], BF16, tag="xrawl", name="xrawl")
    nc.vector.tensor_sub(raw_l, xq, raw_h)
    nc.sync.dma_start(out=xa[0:3, :], in_=sq_h[:, :])
    nc.scalar.dma_start(out=xa[18:21, :], in_=sq_h[:, :])
    nc.sync.dma_start(out=xa[6:9, :], in_=raw_h[:, :])
    nc.scalar.dma_start(out=xa[24:27, :], in_=raw_h[:, :])
    nc.sync.dma_start(out=xa[9:12, :], in_=sq_l[:, :])
    nc.scalar.dma_start(out=xa[15:18, :], in_=raw_l[:, :])
    return xa


def _stage_y_half(nc, pools, y_dram, ya, hh):
    """ya row blocks: [1,y^2_h,-2y_h | 1,y^2_h,-2y_h | 0,y^2_l,-2y_l].
    Uses the scalar HWDGE queue so it doesn't serialize behind x staging."""
    s96 = pools["stage"]
    yw = N // 8
    hs = slice(hh * HALF, (hh + 1) * HALF)
    yq = s96.tile([12, yw], F32, tag="yq", name=f"yq{hh}")
    nc.scalar.dma_start(out=yq, in_=y_dram[:, hs].rearrange("d (c f) -> d c f", c=4))
    ysq = s96.tile([12, yw], F32, tag="ysq", name=f"ysq{hh}")
    nc.scalar.square(ysq, yq)
    ym2 = s96.tile([12, yw], F32, tag="ym2", name=f"ym2{hh}")
    nc.vector.tensor_scalar_mul(ym2, yq, -2.0)
    ysq_h = s96.tile([12, yw], BF16, tag="ysqh", name=f"ysqh{hh}")
    nc.scalar.copy(ysq_h, ysq)
    ysq_l = s96.tile([12, yw], BF16, tag="ysql", name=f"ysql{hh}")
    nc.vector.tensor_sub(ysq_l, ysq, ysq_h)
    ym2_h = s96.tile([12, yw], BF16, tag="ym2h", name=f"ym2h{hh}")
    nc.scalar.copy(ym2_h, ym2)
    ym2_l = s96.tile([12, yw], BF16, tag="ym2l", name=f"ym2l{hh}")
    nc.vector.tensor_sub(ym2_l, ym2, ym2_h)
    eng_a = nc.scalar if hh == 0 else nc.sync
    eng_b = nc.sync
    eng_a.dma_start(out=ya[3:6, hs], in_=ysq_h[:, :])
    eng_b.dma_start(out=ya[12:15, hs], in_=ysq_h[:, :])
    eng_a.dma_start(out=ya[6:9, hs], in_=ym2_h[:, :])
    eng_b.dma_start(out=ya[15:18, hs], in_=ym2_h[:, :])
    eng_a.dma_start(out=ya[21:24, hs], in_=ysq_l[:, :])
    eng_b.dma_start(out=ya[24:27, hs], in_=ym2_l[:, :])


def _huber_sum_into(nc, pools, vals, gal_cols, col, scale=1.0):
    """gal_cols[0:p, col] = scale * sum_free huber(vals); vals f32 [p, W]."""
    small = pools["small"]
    p, w = vals.shape
    hub_m = small.tile([p, w], F32, tag="hub", bufs=4, name=f"hubm{col}")
    hub_r = small.tile([p, w], F32, tag="hub", bufs=4, name=f"hubr{col}")
    nc.vector.tensor_scalar(
        out=hub_m, in0=vals, scalar1=MARGIN, scalar2=float(np.sqrt(0.5)),
        op0=ALU.min, op1=ALU.mult,
    )
    nc.vector.tensor_scalar(
        out=hub_r, in0=vals, scalar1=MARGIN, scalar2=0.0,
        op0=ALU.subtract, op1=ALU.max,
    )
    hub_m2 = small.tile([p, w], F32, tag="hub", bufs=4, name=f"hubm2{col}")
    nc.vector.tensor_mul(hub_m2, hub_m, hub_m)
    hub_full = small.tile([p, w], F32, tag="hub", bufs=4, name=f"hubf{col}")
    nc.vector.scalar_tensor_tensor(
        out=hub_full, in0=hub_r, scalar=MARGIN, in1=hub_m2,
        op0=ALU.mult, op1=ALU.add,
    )
    if scale != 1.0:
        nc.vector.tensor_scalar_mul(hub_full, hub_full, scale)
    nc.vector.reduce_sum(gal_cols[0:p, col : col + 1], hub_full,
                         axis=mybir.AxisListType.X)


def _build(kinv: float):
    nc = bacc.Bacc("TRN2", target_bir_lowering=False, debug=False,
                   num_devices=NCORES)

    pa_x = nc.dram_tensor("pa_x", [3, HALF], F32, kind="ExternalInput").ap()
    pa_y = nc.dram_tensor("pa_y", [3, N], F32, kind="ExternalInput").ap()
    kp_src4 = nc.dram_tensor("kp_src4", [4, KP_H], F32, kind="ExternalInput").ap()
    kp_tgt = nc.dram_tensor("kp_tgt", [3, KP_H], F32, kind="ExternalInput").ap()
    rt4 = nc.dram_tensor("rt4", [4, 3], F32, kind="ExternalInput").ap()
    knn_both = nc.dram_tensor("knn_both", [96, 2 * KP_H], F32,
                              kind="ExternalInput").ap()
    # host-built constant rows DMA'd straight into xa/ya: xa blocks get
    # [ones;zeros;ones], ya blocks [ones;ones;zeros]
    xa_c = nc.dram_tensor("xa_c", [9, HALF], BF16, kind="ExternalInput").ap()
    ya_c = nc.dram_tensor("ya_c", [9, N], BF16, kind="ExternalInput").ap()
    ident_in = nc.dram_tensor("ident", [128, 128], F16, kind="ExternalInput").ap()
    part = nc.dram_tensor("part", [1, 2], F32, kind="ExternalOutput").ap()

    with ExitStack() as ctx:
        tc = ctx.enter_context(tile.TileContext(nc))
        pools = {
            "aug": ctx.enter_context(tc.tile_pool(name="aug", bufs=1)),
            "psum": ctx.enter_context(tc.tile_pool(name="psum", bufs=2, space="PSUM")),
            "stage": ctx.enter_context(tc.tile_pool(name="stage", bufs=2)),
            "scr": ctx.enter_context(tc.tile_pool(name="scr", bufs=3)),
            "small": ctx.enter_context(tc.tile_pool(name="small", bufs=2)),
            "consts": ctx.enter_context(tc.tile_pool(name="consts", bufs=1)),
            "dram": ctx.enter_context(tc.tile_pool(name="dram", bufs=1, space="DRAM")),
        }
        consts, small, scr, psum = (
            pools["consts"], pools["small"], pools["scr"], pools["psum"]
        )

        ones = consts.tile([128, 1], F32)
        nc.vector.memset(ones, 1.0)
        idt = consts.tile([128, 128], F16, name="idt")
        nc.sync.dma_start(out=idt, in_=ident_in)
        xa = _stage_x(nc, pools, pa_x, xa_c)
        ya = pools["aug"].tile([K_ROWS, N], BF16, name="ya")
        _stage_y_half(nc, pools, pa_y, ya, 0)
        nc.gpsimd.dma_start(ya[0:3, :], ya_c[0:3, :])
        nc.gpsimd.dma_start(ya[9:12, :], ya_c[3:6, :])
        nc.gpsimd.dma_start(ya[18:21, :], ya_c[6:9, :])

        # small-loss inputs on the gpsimd queue (off the critical path)
        kps4 = small.tile([4, KP_H], F32, tag="kp", name="kps4")
        kpt = small.tile([3, KP_H], F32, tag="kp", name="kpt")
        rt = small.tile([4, 3], F32, tag="rt")
        knb = small.tile([96, 2 * KP_H], F32, tag="knn", name="knb")
        nc.gpsimd.dma_start(kps4, kp_src4[:, :])
        nc.gpsimd.dma_start(kpt, kp_tgt[:, :])
        nc.gpsimd.dma_start(rt, rt4[:, :])
        nc.gpsimd.dma_start(knb, knn_both[:, :])

        colacc = consts.tile([128, N], F16, name="colacc")
        rowcols = consts.tile([IT, 32], F32, name="rowcols")
        gal_cols = consts.tile([128, 8], F32, name="gal_cols")
        nc.vector.memset(gal_cols, 0.0)

        cmall = consts.tile([128, 32], F32, name="cmall")
        dram = pools["dram"]
        cin = [dram.tile([128, 16], F32, name=f"cin{h}") for h in range(2)]
        cout = [dram.tile([2, 128, 16], F32, name=f"cout{h}") for h in range(2)]

        def tail_half(h):
            """colacc[:, h*2048:(h+1)*2048] -> [128,16] colmin partials ->
            pair ReduceScatter(min) -> huber this core's [64,16] shard."""
            tp32 = psum.tile([IT, GW], F32, tag="ps", name=f"tp{h}")
            tp16 = tp32.bitcast(F16)
            for k in range(16):
                j0 = h * GW + k * 128
                nc.tensor.transpose(
                    tp16[:, k * 128 : (k + 1) * 128],
                    colacc[:, j0 : j0 + 128], idt,
                )
            cm = small.tile([128, 16], F32, tag=f"cm{h}", name=f"cm{h}")
            nc.vector.tensor_reduce(
                out=cm, in_=tp16[:, 0:2048].rearrange("p (k f) -> p k f", k=16),
                axis=mybir.AxisListType.X, op=ALU.min,
            )
            nc.gpsimd.dma_start(cin[h][:, :], cm)
            nc.gpsimd.collective_compute(
                "AllGather", ALU.bypass,
                replica_groups=[[0, 1], [2, 3], [4, 5], [6, 7]],
                ins=[cin[h].opt()], outs=[cout[h].opt()],
            )
            back = small.tile([128, 32], F32, tag=f"cb{h}", name=f"back{h}")
            nc.gpsimd.dma_start(back[:, 0:16], cout[h][0])
            nc.gpsimd.dma_start(back[:, 16:32], cout[h][1])
            nc.vector.tensor_tensor(out=cmall[:, h * 16 : (h + 1) * 16],
                                    in0=back[:, 0:16],
                                    in1=back[:, 16:32], op=ALU.min)

        # ---- main loop ----
        for h in range(2):
            for t in range(N_ITILES):
                lhsT = xa[:, t * IT : (t + 1) * IT]
                j0 = h * GW
                ps = psum.tile([IT, GW], F32, tag="ps", name=f"ps{h}_{t}")
                for mh in range(4):
                    nc.tensor.matmul(
                        out=ps[:, mh * 512 : (mh + 1) * 512], lhsT=lhsT,
                        rhs=ya[:, j0 + mh * 512 : j0 + (mh + 1) * 512],
                        start=True, stop=True)
                st = scr.tile([IT, GW], F16, tag="st", name=f"st{h}_{t}")
                nc.scalar.copy(st, ps)
                # rowmin fold tree (fp16 TT at 2x) + one CR-512
                f1 = scr.tile([IT, 1024], F16, tag="f1", name=f"f1_{h}_{t}")
                nc.vector.tensor_tensor(out=f1, in0=st[:, 0:1024],
                                        in1=st[:, 1024:2048], op=ALU.min)
                f2 = scr.tile([IT, 512], F16, tag="f2", name=f"f2_{h}_{t}")
                nc.vector.tensor_tensor(out=f2, in0=f1[:, 0:512],
                                        in1=f1[:, 512:1024], op=ALU.min)
                dp = scr.tile([IT, 512], F16, tag="dp", name=f"dp{h}_{t}")
                nc.vector.tensor_scalar(
                    out=dp, in0=f2, scalar1=1.0, scalar2=None,
                    op0=ALU.mult, op1=ALU.min,
                    accum_out=rowcols[:, h * 16 + t : h * 16 + t + 1],
                )
                # colacc: first tile of each half initializes by copy
                if t == 0:
                    nc.vector.tensor_copy(colacc[:, j0 : j0 + GW], st)
                else:
                    nc.vector.tensor_tensor(
                        out=colacc[:, j0 : j0 + GW], in0=st,
                        in1=colacc[:, j0 : j0 + GW], op=ALU.min,
                    )
                if h == 0 and t == 1:
                    _stage_y_half(nc, pools, pa_y, ya, 1)
                if h == 0 and t == 3:
                    # keypoint + knn losses early (fills engine slack)
                    kp_full = psum.tile([IT, GW], F32, tag="ps",
                                        name="kp_ps_slot")
                    kp_ps = kp_full[0:3, 0:KP_H]
                    nc.tensor.matmul(out=kp_ps, lhsT=rt, rhs=kps4, start=True,
                                     stop=True)
                    kp_d = small.tile([3, KP_H], F32, tag="kpd", name="kp_d")
                    nc.vector.tensor_sub(kp_d, kp_ps, kpt)
                    kp_sq = small.tile([3, KP_H], F32, tag="kpd", name="kp_sq")
                    nc.vector.tensor_mul(kp_sq, kp_d, kp_d)
                    kp_col = consts.tile([3, 1], F32)
                    nc.vector.reduce_sum(kp_col, kp_sq,
                                         axis=mybir.AxisListType.X)
                    kd = small.tile([96, KP_H], F32, tag="knnd", name="kd")
                    nc.vector.tensor_sub(kd, knb[:, 0:KP_H], knb[:, KP_H:])
                    kd_sq = small.tile([96, KP_H], F32, tag="knnd",
                                       name="kd_sq")
                    nc.vector.tensor_mul(kd_sq, kd, kd)
                    knn_col = consts.tile([96, 1], F32)
                    nc.vector.reduce_sum(knn_col, kd_sq,
                                         axis=mybir.AxisListType.X)
                    nc.vector.tensor_scalar_mul(knn_col, knn_col, kinv)
                if h == 1 and t == 1:
                    tail_half(0)
            if h == 1:
                # rowmin combine + huber first: overlaps the collective wait
                rowmins = consts.tile([IT, N_ITILES], F32, name="rowmins")
                nc.vector.tensor_tensor(out=rowmins, in0=rowcols[:, 0:16],
                                        in1=rowcols[:, 16:32], op=ALU.min)
                _huber_sum_into(nc, pools, rowmins, gal_cols, 0)
                tail_half(1)
                # both pair cores compute the identical full loss_1 share
                _huber_sum_into(nc, pools, cmall, gal_cols, 2, 0.5)

        # ---- final scalar sums ----
        gal_col = consts.tile([128, 1], F32)
        nc.vector.reduce_sum(gal_col, gal_cols, axis=mybir.AxisListType.X)
        fin = psum.tile([IT, GW], F32, tag="ps", name="fin_slot")
        ncl_ps = fin[0:1, 0:1]
        gal_ps = fin[0:1, 4:5]
        nc.tensor.matmul(out=ncl_ps, lhsT=knn_col, rhs=ones[0:96, :],
                         start=True, stop=False)
        nc.tensor.matmul(out=ncl_ps, lhsT=kp_col, rhs=ones[0:3, :],
                         start=False, stop=True)
        nc.tensor.matmul(out=gal_ps, lhsT=gal_col, rhs=ones[:, :],
                         start=True, stop=True)

        outsb = consts.tile([1, 2], F32)
        nc.scalar.copy(outsb[:, 0:1], ncl_ps)
        nc.scalar.copy(outsb[:, 1:2], gal_ps)
        nc.sync.dma_start(out=part[:, :], in_=outsb)

    nc.compile()
    return nc


def _get_nc(kinv: float):
    key = round(kinv, 12)
    if key not in _BUILD_CACHE:
        _BUILD_CACHE[key] = _build(kinv)
    return _BUILD_CACHE[key]


def make_in_maps(src_keypoints, tgt_keypoints, rotation_ab, translation_ab,
                 src_keypoints_knn, tgt_keypoints_knn, src_transformed, tgt):
    a = lambda x: np.ascontiguousarray(np.asarray(x, dtype=np.float32))
    ones_row = np.ones((1, KP_H), dtype=np.float32)
    ident = np.eye(128, dtype=np.float16)
    import ml_dtypes
    xa_c = np.concatenate([
        np.ones((3, HALF)), np.zeros((3, HALF)), np.ones((3, HALF)),
    ]).astype(ml_dtypes.bfloat16)
    ya_c = np.concatenate([
        np.ones((6, N)), np.zeros((3, N)),
    ]).astype(ml_dtypes.bfloat16)
    in_maps = []
    for c in range(NCORES):
        b, h = c // 2, c % 2
        sl = slice(h * HALF, (h + 1) * HALF)
        kpsl = slice(h * KP_H, (h + 1) * KP_H)
        knn_s = (np.asarray(src_keypoints_knn)[b][:, kpsl, :]
                 .transpose(0, 2, 1).reshape(96, KP_H))
        knn_t = (np.asarray(tgt_keypoints_knn)[b][:, kpsl, :]
                 .transpose(0, 2, 1).reshape(96, KP_H))
        in_maps.append({
            "pa_x": a(tgt[b][:, sl]),
            "pa_y": a(src_transformed[b]),
            "kp_src4": a(np.concatenate(
                [ones_row, np.asarray(src_keypoints)[b][:, kpsl]], axis=0)),
            "kp_tgt": a(tgt_keypoints[b][:, kpsl]),
            "rt4": a(np.concatenate([
                np.asarray(translation_ab)[b][None, :],
                np.asarray(rotation_ab)[b].T,
            ], axis=0)),
            "knn_both": a(np.concatenate([knn_s, knn_t], axis=1)),
            "xa_c": xa_c,
            "ya_c": ya_c,
            "ident": ident,
        })
    return in_maps


_RUNNER_CACHE: dict = {}


def _get_runner(kinv: float):
    key = round(kinv, 12)
    if key in _RUNNER_CACHE:
        return _RUNNER_CACHE[key]

    import jax
    from jax.experimental.shard_map import shard_map
    from jax.sharding import Mesh, PartitionSpec
    import concourse.bass2jax as bass2jax
    import concourse.mybir as _mb

    nc = _get_nc(kinv)
    bass2jax.install_neuronx_cc_hook()

    part_name = nc.partition_id_tensor.name if nc.partition_id_tensor else None
    in_names, out_names, out_avals = [], [], []
    for alloc in nc.m.functions[0].allocations:
        if not isinstance(alloc, _mb.MemoryLocationSet):
            continue
        name = alloc.memorylocations[0].name
        if alloc.kind == "ExternalInput":
            if name != part_name:
                in_names.append(name)
        elif alloc.kind == "ExternalOutput":
            out_names.append(name)
            out_avals.append(
                jax.core.ShapedArray(
                    tuple(alloc.tensor_shape), _mb.dt.np(alloc.dtype)
                )
            )
    n_params = len(in_names)
    all_in_names = in_names + out_names
    if part_name is not None:
        all_in_names = all_in_names + [part_name]

    def _body(*args):
        operands = list(args)
        if part_name is not None:
            operands.append(bass2jax.partition_id_tensor())
        outs = bass2jax._bass_exec_p.bind(
            *operands,
            out_avals=tuple(out_avals),
            in_names=tuple(all_in_names),
            out_names=tuple(out_names),
            lowering_input_output_aliases=(),
            sim_require_finite=True,
            sim_require_nnan=True,
            nc=nc,
        )
        return tuple(outs)

    devices = jax.devices()[:NCORES]
    mesh = Mesh(np.asarray(devices), ("core",))
    n_outs = len(out_names)
    sharded = jax.jit(
        shard_map(
            _body,
            mesh=mesh,
            in_specs=(PartitionSpec("core"),) * (n_params + n_outs),
            out_specs=(PartitionSpec("core"),) * n_outs,
            check_rep=False,
        ),
        donate_argnums=tuple(range(n_params, n_params + n_outs)),
        keep_unused=True,
    )

    def run(in_maps):
        concat_in = [
            np.concatenate([m[name] for m in in_maps], axis=0) for name in in_names
        ]
        concat_zeros = [
            np.zeros((NCORES * a.shape[0], *a.shape[1:]), a.dtype) for a in out_avals
        ]
        out_arrs = sharded(*concat_in, *concat_zeros)
        return [
            {
                name: np.asarray(out_arrs[i]).reshape(
                    NCORES, *out_avals[i].shape
                )[c]
                for i, name in enumerate(out_names)
            }
            for c in range(NCORES)
        ]

    _RUNNER_CACHE[key] = run
    return run


def kernel(src_keypoints, tgt_keypoints, rotation_ab, translation_ab,
           src_keypoints_knn, tgt_keypoints_knn, k, src_transformed, tgt,
           _trace=False):
    k_val = float(np.asarray(k))
    in_maps = make_in_maps(
        src_keypoints, tgt_keypoints, rotation_ab, translation_ab,
        src_keypoints_knn, tgt_keypoints_knn, src_transformed, tgt,
    )
    if _trace:
        import shutil
        shutil.rmtree("/tmp/v21_ntff", ignore_errors=True)
        nc = _get_nc(1.0 / k_val)
        res = bass_utils.run_bass_kernel_spmd(
            nc, in_maps, core_ids=list(range(NCORES)), trace=True,
            tmpdir="/tmp/v21_ntff",
        )
        results = res.results
    else:
        run = _get_runner(1.0 / k_val)
        results = run(in_maps)
        res = None
    parts = np.stack([r["part"] for r in results])  # [8, 1, 2]
    ncl = parts[:, 0, 0].astype(np.float64).sum()
    gal = parts[:, 0, 1].astype(np.float64).sum()
    out = (np.float32(ncl), np.float32(gal))
    if _trace:
        return out, res
    return out



# revision 9
# speedup vs baseline: 1.1060x; 1.1060x over previous
"""Trainium2 Bass kernel v3 for nn_LossFunction_49615462203607.

Sharding: core c -> batch b = c//2, j-shard h = c%2 (j = src columns).
Each core computes P[i=tgt 0:4096, j = its 2048-col src shard]:
  - rowmin_i over local j: final per step (one TTR per i-tile), pair
    AllReduce(min) combines the two j-shards; each pair core hubers a
    disjoint partition half.
  - colmin_j: local colacc min across 32 i-tile steps, then 16 PE
    transposes + reduce + huber, fully local (j shards disjoint).

Per step [128 i x 2048 j]:
  PE:  4 row-tiled concurrent matmuls (K=27 bf16 split, tiles at
       partitions 0/32/64/96) -> 4 PSUM banks f32
  Sc:  ACTIVATE copy PSUM f32 -> SBUF fp16 (only PSUM extraction)
  DVE: tensor_tensor_reduce (fold 2048->1024 min + accum rowmin) +
       colacc tensor_tensor min (fp16 2x)

All augmented inputs (squares, bf16 hi/lo splits) are precomputed on the
host: no on-device staging. Rowmin chunk0 collective overlaps steps
16-31; the colmin tail overlaps the chunk1 collective.
"""

import os
from contextlib import ExitStack

import numpy as np

import concourse.bacc as bacc
import concourse.bass as bass
import concourse.mybir as mybir
import concourse.tile as tile
from concourse import bass_utils


F32 = mybir.dt.float32
F16 = mybir.dt.float16
BF16 = mybir.dt.bfloat16
ALU = mybir.AluOpType
AX = mybir.AxisListType

B = 4
N = 4096
KP = 512
MARGIN = 0.01
NCORES = 8
JW = N // 2              # j columns per core (src shard)
IT = 128
N_ITILES = N // IT       # 32 i-tile steps
KP_H = KP // 2
K_ROWS = 27
BIG = 3.0e38

_BUILD_CACHE: dict = {}


def _huber_sum_into(nc, pools, vals, gal_cols, col, p0, p):
    """gal_cols[p0:p0+p, col] += nothing; writes sum_free huber(vals).
    vals f32 [p, W] at base partition p0."""
    small = pools["small"]
    w = vals.shape[-1]
    hub_m = small.tile([128, w], F32, tag="hub", bufs=4, name=f"hubm{col}")
    hub_r = small.tile([128, w], F32, tag="hub", bufs=4, name=f"hubr{col}")
    sl = slice(p0, p0 + p)
    nc.vector.tensor_scalar(
        out=hub_m[sl], in0=vals, scalar1=MARGIN, scalar2=float(np.sqrt(0.5)),
        op0=ALU.min, op1=ALU.mult,
    )
    nc.vector.tensor_scalar(
        out=hub_r[sl], in0=vals, scalar1=MARGIN, scalar2=0.0,
        op0=ALU.subtract, op1=ALU.max,
    )
    hub_m2 = small.tile([128, w], F32, tag="hub", bufs=4, name=f"hubm2{col}")
    nc.vector.tensor_mul(hub_m2[sl], hub_m[sl], hub_m[sl])
    hub_full = small.tile([128, w], F32, tag="hub", bufs=4, name=f"hubf{col}")
    nc.vector.scalar_tensor_tensor(
        out=hub_full[sl], in0=hub_r[sl], scalar=MARGIN, in1=hub_m2[sl],
        op0=ALU.mult, op1=ALU.add,
    )
    nc.vector.reduce_sum(gal_cols[sl, col : col + 1], hub_full[sl], axis=AX.X)


def _build(kinv: float):
    nc = bacc.Bacc("TRN2", target_bir_lowering=False, debug=False,
                   num_devices=NCORES)

    xa_in = nc.dram_tensor("xa_in", [K_ROWS, N], BF16, kind="ExternalInput").ap()
    ya_in = nc.dram_tensor("ya_in", [K_ROWS, JW], BF16, kind="ExternalInput").ap()
    kp_src4 = nc.dram_tensor("kp_src4", [4, KP_H], F32, kind="ExternalInput").ap()
    kp_tgt = nc.dram_tensor("kp_tgt", [3, KP_H], F32, kind="ExternalInput").ap()
    rt4 = nc.dram_tensor("rt4", [4, 3], F32, kind="ExternalInput").ap()
    knn_both = nc.dram_tensor("knn_both", [96, 2 * KP_H], F32,
                              kind="ExternalInput").ap()
    ident_in = nc.dram_tensor("ident", [128, 128], F16, kind="ExternalInput").ap()
    part = nc.dram_tensor("part", [1, 2], F32, kind="ExternalOutput").ap()

    with ExitStack() as ctx:
        tc = ctx.enter_context(tile.TileContext(nc))
        pools = {
            "aug": ctx.enter_context(tc.tile_pool(name="aug", bufs=1)),
            "psum": ctx.enter_context(tc.tile_pool(name="psum", bufs=2, space="PSUM")),
            "scr": ctx.enter_context(tc.tile_pool(name="scr", bufs=3)),
            "small": ctx.enter_context(tc.tile_pool(name="small", bufs=2)),
            "consts": ctx.enter_context(tc.tile_pool(name="consts", bufs=1)),
            "dram": ctx.enter_context(tc.tile_pool(name="dram", bufs=1, space="DRAM")),
        }
        consts, small, scr, psum = (
            pools["consts"], pools["small"], pools["scr"], pools["psum"]
        )

        # ---- load inputs (no staging compute; all precomputed on host) ----
        xa4 = pools["aug"].tile([128, N], BF16, name="xa4")
        ya4 = pools["aug"].tile([128, JW // 4], BF16, name="ya4")
        # replicate xa to partition blocks 0/32/64/96 (row-tiling operands)
        nc.sync.dma_start(out=xa4[0:K_ROWS, :], in_=xa_in)
        nc.scalar.dma_start(out=xa4[32 : 32 + K_ROWS, :], in_=xa_in)
        nc.sync.dma_start(out=xa4[64 : 64 + K_ROWS, :], in_=xa_in)
        nc.scalar.dma_start(out=xa4[96 : 96 + K_ROWS, :], in_=xa_in)
        # ya chunk r lives on partition block r
        for r in range(4):
            eng = nc.sync if r % 2 == 0 else nc.scalar
            eng.dma_start(
                out=ya4[32 * r : 32 * r + K_ROWS, :],
                in_=ya_in[:, r * (JW // 4) : (r + 1) * (JW // 4)],
            )
        idt = consts.tile([128, 128], F16, name="idt")
        nc.gpsimd.dma_start(out=idt, in_=ident_in)
        ones = consts.tile([128, 1], F32)
        nc.vector.memset(ones, 1.0)

        kps4 = small.tile([4, KP_H], F32, tag="kp", name="kps4")
        kpt = small.tile([3, KP_H], F32, tag="kp", name="kpt")
        rt = small.tile([4, 3], F32, tag="rt")
        knb = small.tile([96, 2 * KP_H], F32, tag="knn", name="knb")
        nc.gpsimd.dma_start(kps4, kp_src4[:, :])
        nc.gpsimd.dma_start(kpt, kp_tgt[:, :])
        nc.gpsimd.dma_start(rt, rt4[:, :])
        nc.gpsimd.dma_start(knb, knn_both[:, :])

        colacc = consts.tile([128, JW], F16, name="colacc")
        rowcols = consts.tile([IT, N_ITILES], F32, name="rowcols")
        gal_cols = consts.tile([128, 8], F32, name="gal_cols")
        nc.vector.memset(gal_cols, 0.0)

        dram = pools["dram"]
        cin = [dram.tile([128, 16], F32, name=f"cin{h}") for h in range(2)]
        cout = [dram.tile([2, 128, 16], F32, name=f"cout{h}") for h in range(2)]

        # Both pair cores get the identical pair-combined rowmins, so each
        # computes the full huber sum scaled by 0.5 (no double count).
        def rowmin_chunk(h):
            """rowcols[:, 16h:16h+16] -> pair AllGather -> min -> huber*0.5."""
            nc.gpsimd.dma_start(cin[h][:, :], rowcols[:, 16 * h : 16 * h + 16])
            nc.gpsimd.collective_compute(
                "AllGather", ALU.bypass,
                replica_groups=[[0, 1], [2, 3], [4, 5], [6, 7]],
                ins=[cin[h].opt()], outs=[cout[h].opt()],
            )
            back = small.tile([128, 32], F32, tag=f"rb{h}", name=f"back{h}")
            nc.gpsimd.dma_start(back[:, 0:16], cout[h][0])
            nc.gpsimd.dma_start(back[:, 16:32], cout[h][1])
            rm = small.tile([128, 16], F32, tag=f"rm{h}", name=f"rm{h}")
            nc.vector.tensor_tensor(out=rm, in0=back[:, 0:16],
                                    in1=back[:, 16:32], op=ALU.min)
            hub = small.tile([128, 16], F32, tag=f"rmh{h}", bufs=4, name=f"rmh{h}")
            nc.vector.tensor_scalar(
                out=hub, in0=rm, scalar1=MARGIN, scalar2=float(np.sqrt(0.5)),
                op0=ALU.min, op1=ALU.mult,
            )
            hub2 = small.tile([128, 16], F32, tag=f"rmh{h}", bufs=4, name=f"rmh2{h}")
            nc.vector.tensor_mul(hub2, hub, hub)
            hubr = small.tile([128, 16], F32, tag=f"rmh{h}", bufs=4, name=f"rmhr{h}")
            nc.vector.tensor_scalar(
                out=hubr, in0=rm, scalar1=MARGIN, scalar2=0.0,
                op0=ALU.subtract, op1=ALU.max,
            )
            hubf = small.tile([128, 16], F32, tag=f"rmh{h}", bufs=4, name=f"rmhf{h}")
            nc.vector.scalar_tensor_tensor(
                out=hubf, in0=hubr, scalar=MARGIN, in1=hub2,
                op0=ALU.mult, op1=ALU.add,
            )
            nc.vector.tensor_scalar_mul(hubf, hubf, 0.5)
            nc.vector.reduce_sum(gal_cols[:, h : h + 1], hubf, axis=AX.X)

        # ---- main loop: 32 i-tile steps ----
        for t in range(N_ITILES):
            ps = psum.tile([IT, JW], F32, tag="ps", name=f"ps{t}")
            for r in range(4):
                nc.tensor.matmul(
                    out=ps[:, r * 512 : (r + 1) * 512],
                    lhsT=xa4[32 * r : 32 * r + K_ROWS, t * IT : (t + 1) * IT],
                    rhs=ya4[32 * r : 32 * r + K_ROWS, :],
                    start=True, stop=True, tile_position=(32 * r, 0),
                )
            st = scr.tile([IT, JW], F16, tag="st", name=f"st{t}")
            nc.scalar.copy(st, ps)
            f1 = scr.tile([IT, JW // 2], F16, tag="f1", bufs=2, name=f"f1_{t}")
            nc.vector.tensor_tensor(out=f1, in0=st[:, 0 : JW // 2],
                                    in1=st[:, JW // 2 : JW], op=ALU.min)
            f2 = scr.tile([IT, JW // 4], F16, tag="f2", bufs=2, name=f"f2_{t}")
            nc.vector.tensor_tensor(out=f2, in0=f1[:, 0 : JW // 4],
                                    in1=f1[:, JW // 4 : JW // 2], op=ALU.min)
            dp = scr.tile([IT, JW // 4], F16, tag="dp", bufs=2, name=f"dp{t}")
            nc.vector.tensor_scalar(
                out=dp, in0=f2, scalar1=1.0, scalar2=None,
                op0=ALU.mult, op1=ALU.min,
                accum_out=rowcols[:, t : t + 1],
            )
            if t == 0:
                nc.vector.tensor_copy(colacc, st)
            else:
                nc.vector.tensor_tensor(out=colacc, in0=st, in1=colacc,
                                        op=ALU.min)
            if t == 16:
                rowmin_chunk(0)

        # ---- tail ----
        # rowmin chunk1 collective first (latency-bound; overlap the rest)
        rowmin_chunk(1)

        # colmin: 16 PE transposes -> PSUM fp16, reduce, huber (all local)
        tp32 = psum.tile([IT, JW], F32, tag="ps", name="tp")
        tp16 = tp32.bitcast(F16)
        for k in range(16):
            nc.tensor.transpose(
                tp16[:, k * 128 : (k + 1) * 128],
                colacc[:, k * 128 : (k + 1) * 128], idt,
            )
        cm = small.tile([128, 16], F32, tag="cm", name="cm")
        nc.vector.tensor_reduce(
            out=cm, in_=tp16[:, 0:2048].rearrange("p (k f) -> p k f", k=16),
            axis=AX.X, op=ALU.min,
        )
        _huber_sum_into(nc, pools, cm, gal_cols, 2, 0, 128)

        # keypoint + knn losses (tiny; PE idle here)
        kp_full = psum.tile([IT, JW], F32, tag="ps", name="kp_ps_slot")
        kp_ps = kp_full[0:3, 0:KP_H]
        nc.tensor.matmul(out=kp_ps, lhsT=rt, rhs=kps4, start=True, stop=True)
        kp_d = small.tile([3, KP_H], F32, tag="kpd", name="kp_d")
        nc.vector.tensor_sub(kp_d, kp_ps, kpt)
        kp_sq = small.tile([3, KP_H], F32, tag="kpd", name="kp_sq")
        nc.vector.tensor_mul(kp_sq, kp_d, kp_d)
        kp_col = consts.tile([3, 1], F32)
        nc.vector.reduce_sum(kp_col, kp_sq, axis=AX.X)
        kd = small.tile([96, KP_H], F32, tag="knnd", name="kd")
        nc.vector.tensor_sub(kd, knb[:, 0:KP_H], knb[:, KP_H:])
        kd_sq = small.tile([96, KP_H], F32, tag="knnd", name="kd_sq")
        nc.vector.tensor_mul(kd_sq, kd, kd)
        knn_col = consts.tile([96, 1], F32)
        nc.vector.reduce_sum(knn_col, kd_sq, axis=AX.X)
        nc.vector.tensor_scalar_mul(knn_col, knn_col, kinv)

        # ---- final scalar sums ----
        gal_col = consts.tile([128, 1], F32)
        nc.vector.reduce_sum(gal_col, gal_cols, axis=AX.X)
        fin = psum.tile([IT, JW], F32, tag="ps", name="fin_slot")
        ncl_ps = fin[0:1, 0:1]
        gal_ps = fin[0:1, 4:5]
        nc.tensor.matmul(out=ncl_ps, lhsT=knn_col, rhs=ones[0:96, :],
                         start=True, stop=False)
        nc.tensor.matmul(out=ncl_ps, lhsT=kp_col, rhs=ones[0:3, :],
                         start=False, stop=True)
        nc.tensor.matmul(out=gal_ps, lhsT=gal_col, rhs=ones[:, :],
                         start=True, stop=True)

        outsb = consts.tile([1, 2], F32)
        nc.scalar.copy(outsb[:, 0:1], ncl_ps)
        nc.scalar.copy(outsb[:, 1:2], gal_ps)
        nc.sync.dma_start(out=part[:, :], in_=outsb)

    nc.compile()
    return nc


def _get_nc(kinv: float):
    key = round(kinv, 12)
    if key not in _BUILD_CACHE:
        _BUILD_CACHE[key] = _build(kinv)
    return _BUILD_CACHE[key]


def _split_bf16(v):
    import ml_dtypes
    v = np.asarray(v, dtype=np.float32)
    hi = v.astype(ml_dtypes.bfloat16)
    lo = (v - hi.astype(np.float32)).astype(ml_dtypes.bfloat16)
    return hi, lo


def _augment_x(x):
    """x [3, n] f32 -> xa [27, n] bf16 (tgt / i side)."""
    import ml_dtypes
    n = x.shape[1]
    sq = (np.asarray(x, np.float32) ** 2)
    sq_h, sq_l = _split_bf16(sq)
    raw_h, raw_l = _split_bf16(x)
    one = np.ones((3, n), dtype=ml_dtypes.bfloat16)
    zero = np.zeros((3, n), dtype=ml_dtypes.bfloat16)
    return np.concatenate([
        sq_h, one, raw_h,
        sq_l, zero, raw_l,
        sq_h, one, raw_h,
    ]).astype(ml_dtypes.bfloat16)


def _augment_y(y):
    """y [3, n] f32 -> ya [27, n] bf16 (src / j side)."""
    import ml_dtypes
    n = y.shape[1]
    sq = (np.asarray(y, np.float32) ** 2)
    sq_h, sq_l = _split_bf16(sq)
    m2 = np.asarray(y, np.float32) * -2.0
    m2_h, m2_l = _split_bf16(m2)
    one = np.ones((3, n), dtype=ml_dtypes.bfloat16)
    zero = np.zeros((3, n), dtype=ml_dtypes.bfloat16)
    return np.concatenate([
        one, sq_h, m2_h,
        one, sq_h, m2_h,
        zero, sq_l, m2_l,
    ]).astype(ml_dtypes.bfloat16)


def make_in_maps(src_keypoints, tgt_keypoints, rotation_ab, translation_ab,
                 src_keypoints_knn, tgt_keypoints_knn, src_transformed, tgt):
    a = lambda x: np.ascontiguousarray(np.asarray(x, dtype=np.float32))
    ones_row = np.ones((1, KP_H), dtype=np.float32)
    ident = np.eye(128, dtype=np.float16)
    in_maps = []
    xa_by_b = [np.ascontiguousarray(_augment_x(np.asarray(tgt)[b]))
               for b in range(B)]
    ya_by_b = [np.ascontiguousarray(_augment_y(np.asarray(src_transformed)[b]))
               for b in range(B)]
    for c in range(NCORES):
        b, h = c // 2, c % 2
        jsl = slice(h * JW, (h + 1) * JW)
        kpsl = slice(h * KP_H, (h + 1) * KP_H)
        knn_s = (np.asarray(src_keypoints_knn)[b][:, kpsl, :]
                 .transpose(0, 2, 1).reshape(96, KP_H))
        knn_t = (np.asarray(tgt_keypoints_knn)[b][:, kpsl, :]
                 .transpose(0, 2, 1).reshape(96, KP_H))
        in_maps.append({
            "xa_in": xa_by_b[b],
            "ya_in": np.ascontiguousarray(ya_by_b[b][:, jsl]),
            "kp_src4": a(np.concatenate(
                [ones_row, np.asarray(src_keypoints)[b][:, kpsl]], axis=0)),
            "kp_tgt": a(tgt_keypoints[b][:, kpsl]),
            "rt4": a(np.concatenate([
                np.asarray(translation_ab)[b][None, :],
                np.asarray(rotation_ab)[b].T,
            ], axis=0)),
            "knn_both": a(np.concatenate([knn_s, knn_t], axis=1)),
            "ident": ident,
        })
    return in_maps


_RUNNER_CACHE: dict = {}


def _get_runner(kinv: float):
    key = round(kinv, 12)
    if key in _RUNNER_CACHE:
        return _RUNNER_CACHE[key]

    import jax
    from jax.experimental.shard_map import shard_map
    from jax.sharding import Mesh, PartitionSpec
    import concourse.bass2jax as bass2jax
    import concourse.mybir as _mb

    nc = _get_nc(kinv)
    bass2jax.install_neuronx_cc_hook()

    part_name = nc.partition_id_tensor.name if nc.partition_id_tensor else None
    in_names, out_names, out_avals = [], [], []
    for alloc in nc.m.functions[0].allocations:
        if not isinstance(alloc, _mb.MemoryLocationSet):
            continue
        name = alloc.memorylocations[0].name
        if alloc.kind == "ExternalInput":
            if name != part_name:
                in_names.append(name)
        elif alloc.kind == "ExternalOutput":
            out_names.append(name)
            out_avals.append(
                jax.core.ShapedArray(
                    tuple(alloc.tensor_shape), _mb.dt.np(alloc.dtype)
                )
            )
    n_params = len(in_names)
    all_in_names = in_names + out_names
    if part_name is not None:
        all_in_names = all_in_names + [part_name]

    def _body(*args):
        operands = list(args)
        if part_name is not None:
            operands.append(bass2jax.partition_id_tensor())
        outs = bass2jax._bass_exec_p.bind(
            *operands,
            out_avals=tuple(out_avals),
            in_names=tuple(all_in_names),
            out_names=tuple(out_names),
            lowering_input_output_aliases=(),
            sim_require_finite=True,
            sim_require_nnan=True,
            nc=nc,
        )
        return tuple(outs)

    devices = jax.devices()[:NCORES]
    mesh = Mesh(np.asarray(devices), ("core",))
    n_outs = len(out_names)
    sharded = jax.jit(
        shard_map(
            _body,
            mesh=mesh,
            in_specs=(PartitionSpec("core"),) * (n_params + n_outs),
            out_specs=(PartitionSpec("core"),) * n_outs,
            check_rep=False,
        ),
        donate_argnums=tuple(range(n_params, n_params + n_outs)),
        keep_unused=True,
    )

    def run(in_maps):
        concat_in = [
            np.concatenate([m[name] for m in in_maps], axis=0) for name in in_names
        ]
        concat_zeros = [
            np.zeros((NCORES * a.shape[0], *a.shape[1:]), a.dtype) for a in out_avals
        ]
        out_arrs = sharded(*concat_in, *concat_zeros)
        return [
            {
                name: np.asarray(out_arrs[i]).reshape(
                    NCORES, *out_avals[i].shape
                )[c]
                for i, name in enumerate(out_names)
            }
            for c in range(NCORES)
        ]

    _RUNNER_CACHE[key] = run
    return run


def kernel(src_keypoints, tgt_keypoints, rotation_ab, translation_ab,
           src_keypoints_knn, tgt_keypoints_knn, k, src_transformed, tgt,
           _trace=False):
    k_val = float(np.asarray(k))
    in_maps = make_in_maps(
        src_keypoints, tgt_keypoints, rotation_ab, translation_ab,
        src_keypoints_knn, tgt_keypoints_knn, src_transformed, tgt,
    )
    if _trace:
        import shutil
        shutil.rmtree("/tmp/v3_ntff", ignore_errors=True)
        nc = _get_nc(1.0 / k_val)
        res = bass_utils.run_bass_kernel_spmd(
            nc, in_maps, core_ids=list(range(NCORES)), trace=True,
            tmpdir="/tmp/v3_ntff",
        )
        results = res.results
    else:
        run = _get_runner(1.0 / k_val)
        results = run(in_maps)
        res = None
    parts = np.stack([r["part"] for r in results])  # [8, 1, 2]
    ncl = parts[:, 0, 0].astype(np.float64).sum()
    gal = parts[:, 0, 1].astype(np.float64).sum()
    out = (np.float32(ncl), np.float32(gal))
    if _trace:
        return out, res
    return out
